# revision 8
# baseline (speedup 1.0000x reference)
"""Trainium2 Bass kernel for nn_MFNetLayerL1 (MF-Net detector layer).

Contract: kernel(**inputs) takes FULL unsharded inputs
  log_qi (32768,4,4) f32, G (32768,8,4) f32, sqrt_2rho (32768,) f32,
  n_var (32768,) f32 (unused by the reference), alpha () f32
and returns the FULL (32768,4,4) f32 output.

Strategy: pure data parallel over 8 NeuronCores (4096 samples each).
On-core layout: samples on SBUF partitions (128/tile), 4 sample-tiles
batched per "supertile" (512 samples) to amortize instruction overhead.

Math restructuring (validated against the jax reference in proto.py):
 - laplace_log_cdf(x) = min(x, ln(2 - exp(-|x|))) - ln2  (exact identity);
   the -8*ln2 from the r-sum is folded into the row update as +alpha*C8
   (group-sums of the probability weights are exactly 1).
 - term[b,r,k] = sum_t rho[b]*G[b,r,t]*syms[digit_t(k)] built as a
   pair-decomposition A(i0,i1) (+) B(i2,i3) broadcast add, r innermost.
 - Final output row j = new_row_j - max(new_row_j); the reference's
   repeated global max-subtractions are exact no-ops after the first.
 - Softmax state (E,Z,R,qn) maintained incrementally; only updated rows
   are recomputed (softmax is shift-invariant, so stale unshifted E rows
   still give identical probabilities up to f32 rounding).
"""

import math
import sys

import numpy as np

for _p in ("/opt/trn_rl_repo", "/root/.axon_site/_ro/trn_rl_repo"):
    if _p not in sys.path:
        sys.path.append(_p)

N_TOTAL = 32768
N_CORES = 8
N_PER_CORE = N_TOTAL // N_CORES          # 4096
P = 128                                  # partitions (samples per sub-tile)
T = 4                                    # sub-tiles per supertile
SUPER = P * T                            # 512 samples per supertile
N_SUPER = N_PER_CORE // SUPER            # 8 supertiles per core

SYMS = [s / math.sqrt(5.0) for s in (-3.0, -1.0, 1.0, 3.0)]
C8 = -8.0 * math.log(2.0)

_compiled = None


def _build(n_per_core=N_PER_CORE):
    import concourse.bacc as bacc
    import concourse.tile as tile
    import concourse.mybir as mybir
    from concourse.mybir import (
        AluOpType as op,
        ActivationFunctionType as act,
        AxisListType as ax,
    )

    f32 = mybir.dt.float32
    nc = bacc.Bacc("TRN2", target_bir_lowering=False, debug=False)

    n_super = n_per_core // SUPER
    lq_dram = nc.dram_tensor("log_qi", [n_per_core, 16], f32, kind="ExternalInput")
    g_dram = nc.dram_tensor("G", [n_per_core, 32], f32, kind="ExternalInput")
    rho_dram = nc.dram_tensor("rho", [n_per_core], f32, kind="ExternalInput")
    alpha_dram = nc.dram_tensor("alpha", [1, 1], f32, kind="ExternalInput")
    out_dram = nc.dram_tensor("out", [n_per_core, 16], f32, kind="ExternalOutput")

    def bc(ap, shape):
        return ap.broadcast_to(shape)

    with tile.TileContext(nc) as tc:
        with (
            tc.tile_pool(name="const", bufs=1) as cpool,
            tc.tile_pool(name="io", bufs=3) as io,
            tc.tile_pool(name="big", bufs=2) as big,
            tc.tile_pool(name="small", bufs=2) as small,
        ):
            # ---- one-time constants ----
            syms = cpool.tile([P, 4], f32)
            for i, s in enumerate(SYMS):
                nc.vector.memset(syms[:, i : i + 1], s)
            two = cpool.tile([P, 1], f32)
            nc.vector.memset(two[:], 2.0)
            al = cpool.tile([P, 3], f32)  # [alpha, 1-alpha, alpha*C8]
            nc.sync.dma_start(
                al[:, 0:1], alpha_dram.ap().partition_broadcast(P).squeeze(2)
            )
            alpha_ap = al[:, 0:1]
            nc.vector.tensor_scalar(al[:, 1:2], alpha_ap, -1.0, 1.0, op.mult, op.add)
            nc.vector.tensor_scalar(al[:, 2:3], alpha_ap, C8, None, op.mult)
            onema_ap = al[:, 1:2]
            ac_ap = al[:, 2:3]

            for s in range(n_super):
                base = s * SUPER
                # ---- loads ----
                gt = io.tile([P, T, 8, 4], f32, tag="gt")
                nc.sync.dma_start(
                    gt[:],
                    g_dram[base : base + SUPER, :].rearrange(
                        "(t p) (r c) -> p t r c", p=P, c=4
                    ),
                )
                rho = io.tile([P, T], f32, tag="rho")
                nc.sync.dma_start(
                    rho[:], rho_dram[base : base + SUPER].rearrange("(t p) -> p t", p=P)
                )
                lq = io.tile([P, T, 4, 4], f32, tag="lq")
                nc.sync.dma_start(
                    lq[:],
                    lq_dram[base : base + SUPER, :].rearrange(
                        "(t p) (j i) -> p t j i", p=P, i=4
                    ),
                )

                # ---- phase 1: logp'[t,k] ----
                sg = small.tile([P, T, 8, 4], f32, tag="sg")
                nc.vector.tensor_mul(
                    sg[:], gt[:], bc(rho[:].unsqueeze(2).unsqueeze(3), [P, T, 8, 4])
                )

                ab = small.tile([P, 2, T, 8, 4, 4], f32, tag="ab")  # [A;B]
                tmp = small.tile([P, 2, T, 8, 4, 4], f32, tag="abtmp")
                # views collapsed to <=3 free axes (TENSOR3D ISA limit)
                s_lo = bc(syms[:].unsqueeze(1), [P, T * 8, 4]).unsqueeze(3).broadcast_to([P, T * 8, 4, 4])
                s_hi = bc(syms[:].unsqueeze(1), [P, T * 8, 4]).unsqueeze(2).broadcast_to([P, T * 8, 4, 4])
                for h in range(2):
                    sg_h = sg[:, :, :, 2 * h].rearrange("p t r -> p (t r)")
                    sg_h1 = sg[:, :, :, 2 * h + 1].rearrange("p t r -> p (t r)")
                    nc.vector.tensor_mul(
                        ab[:, h].rearrange("p t r i j -> p (t r) i j"),
                        sg_h.unsqueeze(2).broadcast_to([P, T * 8, 16]),
                        s_lo,
                    )
                    nc.vector.tensor_mul(
                        tmp[:, h].rearrange("p t r i j -> p (t r) i j"),
                        sg_h1.unsqueeze(2).broadcast_to([P, T * 8, 16]),
                        s_hi,
                    )
                nc.vector.tensor_add(
                    ab[:].rearrange("p h t r i j -> p (h t r i j)"),
                    ab[:].rearrange("p h t r i j -> p (h t r i j)"),
                    tmp[:].rearrange("p h t r i j -> p (h t r i j)"),
                )

                # term[t, i01, i23, r] = A[t,r,i01] + B[t,r,i23]
                term = big.tile([P, T, 16, 16, 8], f32, tag="term")
                for t in range(T):
                    a_view = (
                        ab[:, 0, t]
                        .rearrange("p r i j -> p (i j) r")
                        .unsqueeze(2)
                        .broadcast_to([P, 16, 16, 8])
                    )
                    b_view = (
                        ab[:, 1, t]
                        .rearrange("p r i j -> p (i j) r")
                        .unsqueeze(1)
                        .broadcast_to([P, 16, 16, 8])
                    )
                    nc.vector.tensor_add(term[:, t], a_view, b_view)

                work = big.tile([P, T, 16, 16, 8], f32, tag="work")
                wf = work[:].rearrange("p t i j r -> p (t i j r)")
                tf = term[:].rearrange("p t i j r -> p (t i j r)")
                nc.scalar.activation(wf, tf, act.Abs)
                nc.scalar.activation(wf, wf, act.Exp, scale=-1.0)
                nc.scalar.activation(wf, wf, act.Ln, bias=two[:], scale=-1.0)
                nc.vector.tensor_tensor(wf, tf, wf, op.min)

                logp = big.tile([P, T, 4, 4, 4, 4], f32, tag="logp")
                nc.vector.reduce_sum(
                    logp[:].rearrange("p t a b c d -> p t (a b c d)"),
                    work[:].rearrange("p t i j r -> p t (i j) r"),
                    axis=ax.X,
                )

                # ---- phase 2: four sequential row updates ----
                E = small.tile([P, T, 4, 4], f32, tag="E")
                Z = small.tile([P, T, 4], f32, tag="Z")
                R = small.tile([P, T, 4], f32, tag="R")
                qn = small.tile([P, T, 4, 4], f32, tag="qn")
                m23 = small.tile([P, T, 4, 4], f32, tag="m23")
                m01 = small.tile([P, T, 4, 4], f32, tag="m01")
                v = big.tile([P, T, 4, 4, 4, 4], f32, tag="v")
                red = small.tile([P, T, 4], f32, tag="red")
                basev = small.tile([P, T, 4], f32, tag="basev")
                newv = small.tile([P, T, 4], f32, tag="newv")
                mx = small.tile([P, T], f32, tag="mx")
                mx4 = small.tile([P, T, 4], f32, tag="mx4")

                nc.scalar.activation(E[:], lq[:], act.Exp)
                nc.vector.reduce_sum(Z[:], E[:], axis=ax.X)
                nc.vector.reciprocal(R[:], Z[:])
                nc.vector.tensor_mul(qn[:], E[:], bc(R[:].unsqueeze(3), [P, T, 4, 4]))

                for xi in range(4):
                    w = small.tile([P, T, 4, 4, 4], f32, tag="w")
                    if xi == 0:
                        nc.vector.tensor_mul(
                            m23[:],
                            bc(qn[:, :, 2].unsqueeze(3), [P, T, 4, 4]),
                            bc(qn[:, :, 3].unsqueeze(2), [P, T, 4, 4]),
                        )
                        nc.vector.tensor_mul(
                            w[:],
                            qn[:, :, 1].unsqueeze(3).broadcast_to([P, T, 4, 16]),
                            bc(m23[:].rearrange("p t a b -> p t (a b)").unsqueeze(2), [P, T, 4, 16]),
                        )
                        wv = w[:].rearrange("p t a b c -> p t a (b c)").unsqueeze(2).broadcast_to([P, T, 4, 4, 16]).rearrange("p t x a y -> p t x (a y)")
                        vred = v[:].rearrange("p t a b c d -> p t a (b c d)")
                    elif xi == 1:
                        nc.vector.tensor_mul(
                            w[:],
                            qn[:, :, 0].unsqueeze(3).broadcast_to([P, T, 4, 16]),
                            bc(m23[:].rearrange("p t a b -> p t (a b)").unsqueeze(2), [P, T, 4, 16]),
                        )
                        wv = w[:].rearrange("p t a b c -> p (t a) (b c)").unsqueeze(2).broadcast_to([P, T * 4, 4, 16])
                        vred = None
                    elif xi == 2:
                        nc.vector.tensor_mul(
                            m01[:],
                            bc(qn[:, :, 0].unsqueeze(3), [P, T, 4, 4]),
                            bc(qn[:, :, 1].unsqueeze(2), [P, T, 4, 4]),
                        )
                        nc.vector.tensor_mul(
                            w[:],
                            bc(m01[:].rearrange("p t a b -> p t (a b)").unsqueeze(3), [P, T, 16, 4]),
                            qn[:, :, 3].unsqueeze(2).broadcast_to([P, T, 16, 4]),
                        )
                        wv = w[:].rearrange("p t a b c -> p (t a b) c").unsqueeze(2).broadcast_to([P, T * 16, 4, 4])
                        vred = None
                    else:
                        nc.vector.tensor_mul(
                            w[:],
                            bc(m01[:].rearrange("p t a b -> p t (a b)").unsqueeze(3), [P, T, 16, 4]),
                            qn[:, :, 2].unsqueeze(2).broadcast_to([P, T, 16, 4]),
                        )
                        wv = w[:].rearrange("p t a b c -> p (t a b c)").unsqueeze(2).broadcast_to([P, T * 64, 4])
                        vred = v[:].rearrange("p t a b c d -> p t d (a b c)")
                    nc.vector.tensor_mul(
                        v[:].rearrange("p t a b c d -> p (t a b c d)"),
                        logp[:].rearrange("p t a b c d -> p (t a b c d)"),
                        wv,
                    )
                    if xi in (0, 3):
                        nc.vector.reduce_sum(red[:], vred, axis=ax.X)
                    elif xi == 1:
                        for t in range(T):
                            nc.vector.reduce_sum(
                                red[:, t],
                                v[:, t].rearrange("p a b c d -> p b a (c d)"),
                                axis=ax.XY,
                            )
                    else:
                        for t in range(T):
                            nc.vector.reduce_sum(
                                red[:, t],
                                v[:, t].rearrange("p a b c d -> p c (a b) d"),
                                axis=ax.XY,
                            )
                    nc.vector.tensor_scalar(
                        basev[:], lq[:, :, xi], onema_ap, ac_ap, op.mult, op.add
                    )
                    nc.vector.scalar_tensor_tensor(
                        newv[:], red[:], alpha_ap, basev[:], op.mult, op.add
                    )
                    nc.vector.reduce_max(mx[:], newv[:], axis=ax.X)
                    nc.vector.tensor_sub(
                        lq[:, :, xi], newv[:], bc(mx[:].unsqueeze(2), [P, T, 4])
                    )
                    if xi == 0:
                        nc.vector.reduce_max(mx4[:], lq[:], axis=ax.X)
                        nc.vector.tensor_sub(
                            lq[:], lq[:], bc(mx4[:].unsqueeze(3), [P, T, 4, 4])
                        )
                    if xi < 3:
                        nc.scalar.activation(E[:, :, xi], lq[:, :, xi], act.Exp)
                        nc.vector.reduce_sum(Z[:, :, xi], E[:, :, xi], axis=ax.X)
                        nc.vector.reciprocal(R[:, :, xi], Z[:, :, xi])
                        nc.vector.tensor_mul(
                            qn[:, :, xi],
                            E[:, :, xi],
                            bc(R[:, :, xi : xi + 1], [P, T, 4]),
                        )

                # ---- store ----
                nc.sync.dma_start(
                    out_dram[base : base + SUPER, :].rearrange(
                        "(t p) (j i) -> p t j i", p=P, i=4
                    ),
                    lq[:],
                )

    nc.compile()
    return nc


def _get_compiled():
    global _compiled
    if _compiled is None:
        _compiled = _build()
    return _compiled


def run(inputs, trace=False, **kw):
    from concourse.bass_utils import run_bass_kernel_spmd

    nc = _get_compiled()
    log_qi = np.ascontiguousarray(
        np.asarray(inputs["log_qi"], dtype=np.float32)
    ).reshape(N_TOTAL, 16)
    G = np.ascontiguousarray(np.asarray(inputs["G"], dtype=np.float32)).reshape(
        N_TOTAL, 32
    )
    rho = np.ascontiguousarray(np.asarray(inputs["sqrt_2rho"], dtype=np.float32))
    alpha = np.asarray(inputs["alpha"], dtype=np.float32).reshape(1, 1)

    in_maps = []
    for c in range(N_CORES):
        sl = slice(c * N_PER_CORE, (c + 1) * N_PER_CORE)
        in_maps.append(
            {"log_qi": log_qi[sl], "G": G[sl], "rho": rho[sl], "alpha": alpha}
        )
    res = run_bass_kernel_spmd(
        nc, in_maps, core_ids=list(range(N_CORES)), trace=trace, **kw
    )
    out = np.concatenate([r["out"] for r in res.results], axis=0).reshape(
        N_TOTAL, 4, 4
    )
    return out, res


def kernel(**inputs) -> np.ndarray:
    out, _ = run(inputs)
    return out


# revision 14
# speedup vs baseline: 1.6107x; 1.6107x over previous
"""Trainium2 Bass kernel for nn_MFNetLayerL1 (MF-Net detector layer).

Contract: kernel(**inputs) takes FULL unsharded inputs
  log_qi (32768,4,4) f32, G (32768,8,4) f32, sqrt_2rho (32768,) f32,
  n_var (32768,) f32 (unused by the reference), alpha () f32
and returns the FULL (32768,4,4) f32 output.

Strategy: pure data parallel over 8 NeuronCores (4096 samples each);
samples ride SBUF partitions (128/sub-tile), T sub-tiles batched per
"supertile" to amortize instruction overhead.

v2 design (validated against f64 numpy reference via proto.kernel_np_v2):
 - term[b,k,r] = rho[b] * sum_t' G[b,r,t'] * syms[digit_t'(k)] is
   ANTISYMMETRIC under k -> 255-k (syms[3-i] = -syms[i]), so the
   Laplace-logCDF chain runs on half the combos (k<128):
   PE matmuls (stationary G^T, moving const W) build the unscaled term
   for k<128; rho is folded into the ACT abs scale and the DVE min via
   scalar_tensor_tensor.
 - laplace_log_cdf(x) = min(x, p) - ln2 with p = ln(2 - exp(-|x|));
   min(-x, p) = p - |x| - min(x, p) exactly, so the complement half is
   logp[255-k] = D[k] - logp[k], D = sum_r (p - |x|). The -8*ln2 is
   folded into the row update as +alpha*C8 (weight group-sums are 1).
 - r-sums are pairwise trees (DVE for f+, GpSimd for d).
 - Phase-2 group-sums use fused tensor_tensor_reduce (one per (t,s)).
 - Final output row j = new_row_j - max(new_row_j); the reference's
   repeated global max-subtractions are exact no-ops after the first.
"""

import math
import sys

import numpy as np

for _p in ("/opt/trn_rl_repo", "/root/.axon_site/_ro/trn_rl_repo"):
    if _p not in sys.path:
        sys.path.append(_p)

N_TOTAL = 32768
N_CORES = 8
N_PER_CORE = N_TOTAL // N_CORES          # 4096
P = 128                                  # partitions (samples per sub-tile)
T = 4                                    # sub-tiles per supertile
SUPER = P * T                            # samples per supertile
K2 = 128                                 # half of the 256 joint combos

SYMS = [s / math.sqrt(5.0) for s in (-3.0, -1.0, 1.0, 3.0)]
C8 = -8.0 * math.log(2.0)

_compiled = None


def build_w() -> np.ndarray:
    """W[(r,t'), (h, r')] = delta_{r,r'} * syms[digit_t'(h)] for h in [0,128)."""
    W = np.zeros((32, K2 * 8), dtype=np.float32)
    for r in range(8):
        for tp in range(4):
            for h in range(K2):
                dig = (h >> (2 * (3 - tp))) & 3
                W[r * 4 + tp, h * 8 + r] = SYMS[dig]
    return W


def _build(n_per_core=N_PER_CORE):
    import concourse.bacc as bacc
    import concourse.tile as tile
    import concourse.mybir as mybir
    from concourse.bass_types import AP
    from concourse.mybir import (
        AluOpType as op,
        ActivationFunctionType as act,
        AxisListType as ax,
    )

    f32 = mybir.dt.float32
    nc = bacc.Bacc("TRN2", target_bir_lowering=False, debug=False)

    n_super = n_per_core // SUPER
    lq_dram = nc.dram_tensor("log_qi", [n_per_core, 16], f32, kind="ExternalInput")
    gt_dram = nc.dram_tensor("GT", [32, n_per_core], f32, kind="ExternalInput")
    rho_dram = nc.dram_tensor("rho", [n_per_core], f32, kind="ExternalInput")
    alpha_dram = nc.dram_tensor("alpha", [1, 1], f32, kind="ExternalInput")
    w_dram = nc.dram_tensor("W", [32, K2 * 8], f32, kind="ExternalInput")
    out_dram = nc.dram_tensor("out", [n_per_core, 16], f32, kind="ExternalOutput")

    def bc(ap, shape):
        return ap.broadcast_to(shape)

    with tile.TileContext(nc) as tc:
        with (
            tc.tile_pool(name="const", bufs=1) as cpool,
            tc.tile_pool(name="io", bufs=3) as io,
            tc.tile_pool(name="psum", bufs=3, space="PSUM") as psum,
            tc.tile_pool(name="big", bufs=2) as big,
            tc.tile_pool(name="small", bufs=2) as small,
        ):
            # ---- one-time constants ----
            w_sb = cpool.tile([32, K2 * 8], f32)
            nc.sync.dma_start(w_sb[:], w_dram[:, :])
            two = cpool.tile([P, 1], f32)
            nc.vector.memset(two[:], 2.0)
            al = cpool.tile([P, 3], f32)  # [alpha, 1-alpha, alpha*C8]
            nc.sync.dma_start(
                al[:, 0:1], alpha_dram.ap().partition_broadcast(P).squeeze(2)
            )
            alpha_ap = al[:, 0:1]
            nc.vector.tensor_scalar(al[:, 1:2], alpha_ap, -1.0, 1.0, op.mult, op.add)
            nc.vector.tensor_scalar(al[:, 2:3], alpha_ap, C8, None, op.mult)
            onema_ap = al[:, 1:2]
            ac_ap = al[:, 2:3]

            for s in range(n_super):
                base = s * SUPER
                # ---- loads ----
                gt = io.tile([32, SUPER], f32, tag="gt")
                nc.sync.dma_start(gt[:], gt_dram[:, base : base + SUPER])
                rho = io.tile([P, T], f32, tag="rho")
                nc.sync.dma_start(
                    rho[:], rho_dram[base : base + SUPER].rearrange("(t p) -> p t", p=P)
                )
                lq = io.tile([P, T, 4, 4], f32, tag="lq")
                nc.sync.dma_start(
                    lq[:],
                    lq_dram[base : base + SUPER, :].rearrange(
                        "(t p) (j i) -> p t j i", p=P, i=4
                    ),
                )

                # ---- phase 1: logp[t, k] for k<128 via PE + f-chain ----
                a_t = big.tile([P, T, K2, 8], f32, tag="a")
                u_t = big.tile([P, T, K2, 8], f32, tag="u")   # exp(-a), then d=p-a
                p_t = big.tile([P, T, K2, 8], f32, tag="p")
                fp_t = big.tile([P, T, K2, 8], f32, tag="fp")
                logp = big.tile([P, T, 4, 4, 4, 4], f32, tag="logp")

                for t in range(T):
                    term_ps = psum.tile([P, K2 * 8], f32, tag="term")
                    for j in range(2):
                        nc.tensor.matmul(
                            term_ps[:, 512 * j : 512 * (j + 1)],
                            gt[:, 128 * t : 128 * (t + 1)],
                            w_sb[:, 512 * j : 512 * (j + 1)],
                            start=True,
                            stop=True,
                        )
                    rho_t = rho[:, t : t + 1]
                    af = a_t[:, t].rearrange("p k r -> p (k r)")
                    uf = u_t[:, t].rearrange("p k r -> p (k r)")
                    pf = p_t[:, t].rearrange("p k r -> p (k r)")
                    ff = fp_t[:, t].rearrange("p k r -> p (k r)")
                    nc.scalar.activation(af, term_ps[:], act.Abs, scale=rho_t)
                    nc.scalar.activation(uf, af, act.Exp, scale=-1.0)
                    nc.scalar.activation(pf, uf, act.Ln, bias=two[:], scale=-1.0)
                    nc.vector.scalar_tensor_tensor(
                        ff, term_ps[:], rho_t, pf, op.mult, op.min
                    )
                    # d = p - a, overwrite u (dead after ln)
                    nc.gpsimd.tensor_sub(uf, pf, af)

                # pairwise r-trees: logp_plus on DVE, D on GpSimd
                s1p = big.tile([P, T * K2 * 4], f32, tag="a")  # reuse dead a slot ring
                s2p = big.tile([P, T * K2 * 2], f32, tag="s2p")
                s1d = big.tile([P, T * K2 * 4], f32, tag="p")  # reuse dead p slot ring
                s2d = big.tile([P, T * K2 * 2], f32, tag="s2d")
                dtot = big.tile([P, T * K2], f32, tag="dtot")

                def halver(eng, out_ap, in_ap, n_pairs):
                    i0 = AP(in_ap.tensor, in_ap.offset, [list(in_ap.ap[0]), [2, n_pairs]])
                    i1 = AP(in_ap.tensor, in_ap.offset + 1, [list(in_ap.ap[0]), [2, n_pairs]])
                    eng.tensor_add(out_ap, i0, i1)

                fp_f = fp_t[:].rearrange("p t k r -> p (t k r)")
                d_f = u_t[:].rearrange("p t k r -> p (t k r)")
                lpf = logp[:].rearrange("p t a b c d -> p t (a b c d)")
                lp_plus = lpf[:, :, 0:K2]
                halver(nc.vector, s1p[:], fp_f, T * K2 * 4)
                halver(nc.vector, s2p[:], s1p[:], T * K2 * 2)
                halver(nc.vector, lp_plus, s2p[:], T * K2)
                halver(nc.gpsimd, s1d[:], d_f, T * K2 * 4)
                halver(nc.gpsimd, s2d[:], s1d[:], T * K2 * 2)
                halver(nc.gpsimd, dtot[:], s2d[:], T * K2)
                # logp[255-h] = D[h] - logp_plus[h]: negative-step write
                lp_minus_rev = AP(
                    lpf.tensor,
                    lpf.offset + 255,
                    [list(lpf.ap[0]), [256, T], [-1, K2]],
                )
                nc.vector.tensor_sub(
                    lp_minus_rev, dtot[:].rearrange("p (t k) -> p t k", t=T), lp_plus
                )

                # ---- phase 2: four sequential row updates ----
                E = small.tile([P, T, 4, 4], f32, tag="E")
                Z = small.tile([P, T, 4], f32, tag="Z")
                R = small.tile([P, T, 4], f32, tag="R")
                qn = small.tile([P, T, 4, 4], f32, tag="qn")
                red = small.tile([P, T, 4], f32, tag="red")
                basev = small.tile([P, T, 4], f32, tag="basev")
                newv = small.tile([P, T, 4], f32, tag="newv")
                mx = small.tile([P, T], f32, tag="mx")
                mx4 = small.tile([P, T, 4], f32, tag="mx4")

                nc.scalar.activation(E[:], lq[:], act.Exp)
                nc.vector.reduce_sum(Z[:], E[:], axis=ax.X)
                nc.vector.reciprocal(R[:], Z[:])
                nc.vector.tensor_mul(qn[:], E[:], bc(R[:].unsqueeze(3), [P, T, 4, 4]))

                cU = small.tile([P, T, 4, 4, 4], f32, tag="cU")
                cV = small.tile([P, T, 4, 4], f32, tag="cV")
                cW = small.tile([P, T, 4, 4], f32, tag="cW")
                cP = small.tile([P, T, 4, 4, 4], f32, tag="cP")
                cP2 = small.tile([P, T, 4, 4], f32, tag="cP2")
                vbig = small.tile([P, T, 256], f32, tag="vbig")
                vmid = small.tile([P, T, 64], f32, tag="vmid")

                def q_over_inner(j, ni):
                    # qn[:, :, j, c] broadcast over ni trailing elements (c outer)
                    return qn[:, :, j].unsqueeze(3).broadcast_to([P, T, 4, ni])

                def q_over_outer(j, no):
                    # qn[:, :, j, c] broadcast over no leading elements (c inner)
                    return qn[:, :, j].unsqueeze(2).broadcast_to([P, T, no, 4])

                vsm = small.tile([P, T, 16], f32, tag="vsm")

                def scr_view(nelem):
                    base = vbig if nelem == 256 else (vmid if nelem == 64 else vsm)
                    return base[:].rearrange("p t (x c) -> p t x c", c=4)

                def contract_inner(out_ap, in0, j, no):
                    # out[x] = sum_c in0[x, c] * q_j[c]; in0 (P,T,no,4)
                    scr = scr_view(no * 4)
                    nc.vector.tensor_mul(scr, in0, q_over_outer(j, no))
                    nc.vector.reduce_sum(out_ap, scr, axis=ax.X)

                def contract_outer(out_ap, in0, j, ni):
                    # out[x] = sum_c in0[c, x] * q_j[c]; in0 (P,T,4,ni)
                    scr = scr_view(ni * 4).rearrange("p t x c -> p t (x c)").rearrange(
                        "p t (c x) -> p t c x", c=4)
                    nc.vector.tensor_mul(scr, in0, q_over_inner(j, ni))
                    nc.vector.reduce_sum(out_ap, scr.transpose([0, 1, 3, 2]), axis=ax.X)

                for xi in range(4):
                    if xi == 0:
                        contract_inner(
                            cU[:].rearrange("p t a b c -> p t (a b) c"),
                            lpf[:].rearrange("p t (x d) -> p t x d", d=4),
                            3, 64,
                        )
                        contract_inner(
                            cV[:].rearrange("p t a b -> p t (a b)"),
                            cU[:].rearrange("p t a b c -> p t (a b) c"),
                            2, 16,
                        )
                        contract_inner(red[:], cV[:], 1, 4)
                    elif xi == 1:
                        contract_outer(red[:], cV[:], 0, 4)
                    elif xi == 2:
                        contract_outer(
                            cW[:].rearrange("p t b c -> p t (b c)"),
                            cU[:].rearrange("p t a b c -> p t a (b c)"),
                            0, 16,
                        )
                        contract_outer(red[:], cW[:], 1, 4)
                    else:
                        contract_outer(
                            cP[:].rearrange("p t b c d -> p t (b c d)"),
                            lpf[:].rearrange("p t (a x) -> p t a x", a=4),
                            0, 64,
                        )
                        contract_outer(
                            cP2[:].rearrange("p t c d -> p t (c d)"),
                            cP[:].rearrange("p t b c d -> p t b (c d)"),
                            1, 16,
                        )
                        contract_outer(red[:], cP2[:], 2, 4)
                    nc.vector.tensor_scalar(
                        basev[:], lq[:, :, xi], onema_ap, ac_ap, op.mult, op.add
                    )
                    nc.vector.scalar_tensor_tensor(
                        newv[:], red[:], alpha_ap, basev[:], op.mult, op.add
                    )
                    nc.vector.reduce_max(mx[:], newv[:], axis=ax.X)
                    nc.vector.tensor_sub(
                        lq[:, :, xi], newv[:], bc(mx[:].unsqueeze(2), [P, T, 4])
                    )
                    if xi == 0:
                        nc.vector.reduce_max(mx4[:], lq[:], axis=ax.X)
                        nc.vector.tensor_sub(
                            lq[:], lq[:], bc(mx4[:].unsqueeze(3), [P, T, 4, 4])
                        )
                    if xi < 3:
                        nc.scalar.activation(E[:, :, xi], lq[:, :, xi], act.Exp)
                        nc.vector.reduce_sum(Z[:, :, xi], E[:, :, xi], axis=ax.X)
                        nc.vector.reciprocal(R[:, :, xi], Z[:, :, xi])
                        nc.vector.tensor_mul(
                            qn[:, :, xi],
                            E[:, :, xi],
                            bc(R[:, :, xi : xi + 1], [P, T, 4]),
                        )

                # ---- store ----
                nc.sync.dma_start(
                    out_dram[base : base + SUPER, :].rearrange(
                        "(t p) (j i) -> p t j i", p=P, i=4
                    ),
                    lq[:],
                )

    nc.compile()
    return nc


def _get_compiled():
    global _compiled
    if _compiled is None:
        _compiled = _build()
    return _compiled


def make_in_map(log_qi, G, rho, alpha):
    """Per-core input map from natural-layout slices."""
    n = log_qi.shape[0]
    return {
        "log_qi": np.ascontiguousarray(np.asarray(log_qi, dtype=np.float32).reshape(n, 16)),
        "GT": np.ascontiguousarray(np.asarray(G, dtype=np.float32).reshape(n, 32).T),
        "rho": np.ascontiguousarray(np.asarray(rho, dtype=np.float32)),
        "alpha": np.asarray(alpha, dtype=np.float32).reshape(1, 1),
        "W": build_w(),
    }


def run(inputs, trace=False, **kw):
    from concourse.bass_utils import run_bass_kernel_spmd

    nc = _get_compiled()
    log_qi = np.asarray(inputs["log_qi"], dtype=np.float32)
    G = np.asarray(inputs["G"], dtype=np.float32)
    rho = np.asarray(inputs["sqrt_2rho"], dtype=np.float32)
    alpha = inputs["alpha"]

    in_maps = []
    for c in range(N_CORES):
        sl = slice(c * N_PER_CORE, (c + 1) * N_PER_CORE)
        in_maps.append(make_in_map(log_qi[sl], G[sl], rho[sl], alpha))
    res = run_bass_kernel_spmd(
        nc, in_maps, core_ids=list(range(N_CORES)), trace=trace, **kw
    )
    out = np.concatenate([r["out"] for r in res.results], axis=0).reshape(
        N_TOTAL, 4, 4
    )
    return out, res


def kernel(**inputs) -> np.ndarray:
    out, _ = run(inputs)
    return out
